# revision 1
# baseline (speedup 1.0000x reference)
"""Multi-head attention (B=2, F=T=2048, H=1024, 16 heads x 64) on 8 TRN2
NeuronCores.

Sharding: pure head/tensor parallelism with an output-side AllToAll.
Core c owns heads {2c, 2c+1} for BOTH batches. Each core:
  1. projects Q^T / K^T / V for its 2 heads over the full sequences
     (both batches, front-loaded so attention owns all 8 PSUM banks),
  2. runs attention for its heads, software-pipelined so the ACT engine
     (exp) is the critical path: S/exp for step i+1 are emitted before
     the P@V matmuls of step i. Softmax denominators come free from a
     ones-column appended to V in the P@V matmul; exp folds the
     1/sqrt(64) logit scale into its free affine,
  3. normalizes A^T with a scale chain (DVE reciprocal -> Kc=1 PE
     ones-broadcast -> DVE multiply) deferred into the next f-chunk's
     loop so the in-order PE never stalls on it; each finished (batch,
     f-chunk) shard is DMA'd straight into the AllToAll input buffer,
  4. one 8-core AllToAll redistributes A^T from head-sharded to
     (batch, query-slice)-sharded, and the output projection runs locally
     with the full 1024-deep head contraction -> exact [512, 1024] slice.
Host concatenates the 8 slices. All matmuls run in bf16 with fp32 PSUM
accumulation.
"""

from contextlib import ExitStack

import ml_dtypes
import numpy as np

import concourse.bass as bass  # noqa: F401
import concourse.mybir as mybir
import concourse.tile as tile
from concourse import bacc
from concourse.bass_utils import run_bass_kernel_spmd

B, F, T, HID, NH, DH = 2, 2048, 2048, 1024, 16, 64
FS = F // 4  # 512-row output slice per core
HT = HID // 128  # 8 h-tiles
TT = T // 128  # 16 key tiles
FC = F // 512  # 4 query chunks
BF16, F32 = mybir.dt.bfloat16, mybir.dt.float32
NPBF16 = ml_dtypes.bfloat16

_CACHE: dict = {}


def _build():
    nc = bacc.Bacc("TRN2", target_bir_lowering=False, debug=False, num_devices=8)

    qT = nc.declare_dram_parameter("qT", [B, HID, F], BF16, isOutput=False)
    sT = nc.declare_dram_parameter("sT", [B, HID, T], BF16, isOutput=False)
    w3 = nc.declare_dram_parameter("w3", [HID, 384], BF16, isOutput=False)
    wo = nc.declare_dram_parameter("wo", [HID, HID], BF16, isOutput=False)
    out = nc.declare_dram_parameter("out", [FS, HID], F32, isOutput=True)

    seg = 128 * FS  # one A^T shard: [128 hd, 512 f]
    a2a_in = nc.dram_tensor("a2a_in", [8, seg], BF16)
    a2a_out = nc.dram_tensor("a2a_out", [8, seg], BF16)

    with tile.TileContext(nc) as tc, ExitStack() as ctx:
        persist = ctx.enter_context(tc.tile_pool(name="persist", bufs=1))
        kT_sb = persist.tile([128, B, T], BF16, tag="kT")
        v_sb = persist.tile([128, B, TT, 2, DH + 1], BF16, tag="v")
        qTp_sb = persist.tile([128, B, F], BF16, tag="qTp")
        wo_sb = persist.tile([128, HT, HID], BF16, tag="wo")
        w3_sb = persist.tile([128, HT, 3, 128], BF16, tag="w3")  # wq|wk|wv
        ones_sb = persist.tile([128, DH, 1], BF16, tag="ones")

        nc.vector.memset(ones_sb[:, :, :], 1.0)
        nc.vector.memset(v_sb[:, :, :, :, DH : DH + 1], 1.0)
        nc.sync.dma_start(
            out=w3_sb[:, :, :, :],
            in_=w3[:, :].rearrange("(a p) (k n) -> p a k n", p=128, n=128),
        )

        with (
            tc.tile_pool(name="inp", bufs=1) as inp_pool,
            tc.tile_pool(name="inps", bufs=2) as inps_pool,
            tc.tile_pool(name="ptp", bufs=6) as pt_pool,
            tc.tile_pool(name="rtp", bufs=4) as rt_pool,
            tc.tile_pool(name="stg", bufs=4) as stg_pool,
        ):
            # ---- projections (both batches, PSUM scope closes after) --
            with tc.tile_pool(name="proj_ps", bufs=2, space="PSUM") as proj_ps:
                for b in range(B):
                    sT_sb = inps_pool.tile([128, HT, T], BF16, tag="sT")
                    nc.sync.dma_start(
                        out=sT_sb[:, :, :],
                        in_=sT[b, :, :].rearrange("(a p) n -> p a n", p=128),
                    )
                    # K^T [128 hd, T]
                    for c in range(T // 512):
                        ps = proj_ps.tile([128, 512], F32, tag="ps")
                        for ht in range(HT):
                            nc.tensor.matmul(
                                ps[:, :],
                                lhsT=w3_sb[:, ht, 1, :],
                                rhs=sT_sb[:, ht, 512 * c : 512 * (c + 1)],
                                start=(ht == 0),
                                stop=(ht == HT - 1),
                            )
                        nc.vector.tensor_copy(
                            out=kT_sb[:, b, 512 * c : 512 * (c + 1)], in_=ps[:, :]
                        )
                    # V [t, 2*DH] per key tile
                    for tt in range(TT):
                        ps = proj_ps.tile([128, 128], F32, tag="ps")
                        for ht in range(HT):
                            nc.tensor.matmul(
                                ps[:, :],
                                lhsT=sT_sb[:, ht, 128 * tt : 128 * (tt + 1)],
                                rhs=w3_sb[:, ht, 2, :],
                                start=(ht == 0),
                                stop=(ht == HT - 1),
                            )
                        nc.vector.tensor_copy(
                            out=v_sb[:, b, tt, :, 0:DH],
                            in_=ps[:, :].rearrange("p (j d) -> p j d", j=2),
                        )
                    # Q^T [128 hd, F]
                    qT_sb = inp_pool.tile([128, HT, F], BF16, tag="qT")
                    nc.sync.dma_start(
                        out=qT_sb[:, :, :],
                        in_=qT[b, :, :].rearrange("(a p) n -> p a n", p=128),
                    )
                    for c in range(FC):
                        ps = proj_ps.tile([128, 512], F32, tag="ps")
                        for ht in range(HT):
                            nc.tensor.matmul(
                                ps[:, :],
                                lhsT=w3_sb[:, ht, 0, :],
                                rhs=qT_sb[:, ht, 512 * c : 512 * (c + 1)],
                                start=(ht == 0),
                                stop=(ht == HT - 1),
                            )
                        nc.vector.tensor_copy(
                            out=qTp_sb[:, b, 512 * c : 512 * (c + 1)], in_=ps[:, :]
                        )

            # ---- attention (both batches); scale chains deferred by one
            # fc so the in-order PE never stalls on recip/broadcast ------
            with (
                tc.tile_pool(name="s_ps", bufs=2, space="PSUM") as s_ps_pool,
                tc.tile_pool(name="a_ps", bufs=2, space="PSUM") as a_ps_pool,
            ):

                def flush_scale(b, fc, a_ps):
                    shard = a2a_in[4 * b + fc, :].rearrange(
                        "(p n) -> p n", p=128
                    )
                    rts = []
                    for j in range(2):
                        rt = rt_pool.tile([65, 1, 512], BF16, tag="rt")
                        with nc.allow_low_precision("bf16 softmax denom recip"):
                            nc.vector.reciprocal(
                                out=rt[64:65, 0, :], in_=a_ps[64:65, j, :]
                            )
                        rts.append(rt)
                    for j in range(2):
                        rt = rts[j]
                        bc = s_ps_pool.tile([64, 512], F32, tag="s")
                        nc.tensor.matmul(
                            bc[:, :],
                            lhsT=ones_sb[64:65, :, 0],
                            rhs=rt[64:65, 0, :],
                            start=True,
                            stop=True,
                        )
                        bc_sb = rt_pool.tile([64, 512], F32, tag="bc")
                        nc.vector.tensor_copy(out=bc_sb[:, :], in_=bc[:, :])
                        st = stg_pool.tile([64, 512], BF16, tag="st")
                        nc.vector.tensor_mul(
                            out=st[:, :], in0=a_ps[0:64, j, :], in1=bc_sb[:, :]
                        )
                        nc.sync.dma_start(
                            out=shard[64 * j : 64 * (j + 1), :], in_=st[:, :]
                        )

                def emit_s_exp(b, fc, tt):
                    sp = s_ps_pool.tile([128, 2, 512], F32, tag="s")
                    for j in range(2):
                        nc.tensor.matmul(
                            sp[:, j, :],
                            lhsT=kT_sb[
                                64 * j : 64 * (j + 1), b, 128 * tt : 128 * (tt + 1)
                            ],
                            rhs=qTp_sb[
                                64 * j : 64 * (j + 1), b, 512 * fc : 512 * (fc + 1)
                            ],
                            start=True,
                            stop=True,
                        )
                    pt = pt_pool.tile([128, 2, 512], BF16, tag="pt")
                    nc.scalar.activation(
                        out=pt[:, :, :],
                        in_=sp[:, :, :],
                        func=mybir.ActivationFunctionType.Exp,
                        scale=float(DH) ** -0.5,
                    )
                    return pt

                # software-pipelined: S/exp run one (b,fc,tt) step ahead of
                # the P@V accumulation so the in-order PE never waits on exp
                steps = [
                    (b, fc, tt) for b in range(B) for fc in range(FC)
                    for tt in range(TT)
                ]
                pending = None
                a_tiles = {}
                pts = {}
                pts[steps[0]] = emit_s_exp(*steps[0])
                for i, (b, fc, tt) in enumerate(steps):
                    if tt == 0:
                        a_tiles[(b, fc)] = a_ps_pool.tile(
                            [65, 2, 512], F32, tag="a", name="a_acc"
                        )
                    if i + 1 < len(steps):
                        pts[steps[i + 1]] = emit_s_exp(*steps[i + 1])
                    a_ps = a_tiles[(b, fc)]
                    pt = pts.pop((b, fc, tt))
                    for j in range(2):
                        nc.tensor.matmul(
                            a_ps[:, j, :],
                            lhsT=v_sb[:, b, tt, j, :],
                            rhs=pt[:, j, :],
                            start=(tt == 0),
                            stop=(tt == TT - 1),
                        )
                    if tt == 10 and pending is not None:
                        flush_scale(*pending)
                        pending = None
                    if tt == TT - 1:
                        pending = (b, fc, a_ps)
                flush_scale(*pending)

        nc.scalar.dma_start(
            out=wo_sb[:, :, :], in_=wo[:, :].rearrange("(a p) n -> p a n", p=128)
        )

        # ---- AllToAll: head-sharded -> (batch, f-slice)-sharded -------
        nc.gpsimd.collective_compute(
            "AllToAll",
            mybir.AluOpType.bypass,
            replica_groups=[[0, 1, 2, 3, 4, 5, 6, 7]],
            ins=[a2a_in.ap().opt()],
            outs=[a2a_out.ap().opt()],
        )

        with (
            tc.tile_pool(name="atg", bufs=1) as atg_pool,
            tc.tile_pool(name="o_ps", bufs=4, space="PSUM") as o_ps_pool,
            tc.tile_pool(name="op", bufs=2) as out_pool,
        ):
            atg_sb = atg_pool.tile([128, HT, FS], BF16, tag="atg")
            nc.sync.dma_start(
                out=atg_sb[:, :, :],
                in_=a2a_out[:, :].rearrange("a (p n) -> p a n", p=128),
            )
            for ft in range(FS // 128):
                o_ps = o_ps_pool.tile([128, 2, 512], F32, tag="o")
                for p in range(HT):
                    for j in range(2):
                        nc.tensor.matmul(
                            o_ps[:, j, :],
                            lhsT=atg_sb[:, p, 128 * ft : 128 * (ft + 1)],
                            rhs=wo_sb[:, p, 512 * j : 512 * (j + 1)],
                            start=(p == 0),
                            stop=(p == HT - 1),
                        )
                ot = out_pool.tile([128, HID], F32, tag="ot")
                nc.vector.tensor_copy(
                    out=ot[:, :].rearrange("p (j n) -> p j n", j=2),
                    in_=o_ps[:, :, :],
                )
                nc.sync.dma_start(
                    out=out[128 * ft : 128 * (ft + 1), :], in_=ot[:, :]
                )

    nc.compile()
    return nc


def _get_nc():
    if "nc" not in _CACHE:
        _CACHE["nc"] = _build()
    return _CACHE["nc"]


def _reference_fallback(query_input, source_input, bias, wq, wk, wv, wo):
    """Numpy fallback, only used if bias is unexpectedly nonzero."""
    q = np.einsum("bfh,hnd->bfnd", query_input, wq) * (DH**-0.5)
    k = np.einsum("bth,hnd->btnd", source_input, wk)
    v = np.einsum("bth,hnd->btnd", source_input, wv)
    logits = np.einsum("btnd,bfnd->bnft", k, q) + bias
    logits -= logits.max(axis=-1, keepdims=True)
    w = np.exp(logits)
    w /= w.sum(axis=-1, keepdims=True)
    attn = np.einsum("bnft,btnd->bfnd", w, v)
    return np.einsum("bfnd,ndh->bfh", attn, wo).astype(np.float32)


def make_in_maps(query_input, source_input, wq, wk, wv, wo):
    wo2 = np.ascontiguousarray(wo.reshape(HID, HID).astype(NPBF16))
    qTb = np.ascontiguousarray(
        np.transpose(query_input, (0, 2, 1))
    ).astype(NPBF16)  # [B, HID, F]
    sTb = np.ascontiguousarray(np.transpose(source_input, (0, 2, 1))).astype(NPBF16)
    wqh = wq.reshape(HID, NH, DH)
    wkh = wk.reshape(HID, NH, DH)
    wvh = wv.reshape(HID, NH, DH)

    in_maps = []
    for c in range(8):
        sl = np.s_[:, 2 * c : 2 * c + 2, :]
        w3c = np.concatenate(
            [
                wqh[sl].reshape(HID, 128),
                wkh[sl].reshape(HID, 128),
                wvh[sl].reshape(HID, 128),
            ],
            axis=1,
        )
        in_maps.append(
            {
                "qT": qTb,
                "sT": sTb,
                "w3": np.ascontiguousarray(w3c).astype(NPBF16),
                "wo": wo2,
            }
        )
    return in_maps


def kernel(query_input, source_input, bias, wq, wk, wv, wo):
    query_input = np.asarray(query_input, dtype=np.float32)
    source_input = np.asarray(source_input, dtype=np.float32)
    bias = np.asarray(bias, dtype=np.float32)
    wq = np.asarray(wq, dtype=np.float32)
    wk = np.asarray(wk, dtype=np.float32)
    wv = np.asarray(wv, dtype=np.float32)
    wo = np.asarray(wo, dtype=np.float32)

    if np.any(bias):
        return _reference_fallback(query_input, source_input, bias, wq, wk, wv, wo)

    in_maps = make_in_maps(query_input, source_input, wq, wk, wv, wo)
    nc = _get_nc()
    res = run_bass_kernel_spmd(nc, in_maps, core_ids=list(range(8)))

    out_full = np.empty((B, F, HID), dtype=np.float32)
    for c in range(8):
        b, r = c // 4, c % 4
        out_full[b, FS * r : FS * (r + 1), :] = res.results[c]["out"]
    return out_full



# revision 11
# speedup vs baseline: 1.0757x; 1.0757x over previous
"""Multi-head attention (B=2, F=T=2048, H=1024, 16 heads x 64) on 8 TRN2
NeuronCores.

v2 design (vs baseline):
  * exp split across engines: per attention step, head j=0's exp runs on
    ACT (exact, free affine scale), head j=1's exp runs on DVE as a
    one-instruction Schraudolph bf16 exp (tensor_scalar mult+add with
    int16 output whose bits ARE the bf16 exp). Halves the softmax
    critical path (ACT was 163us busy in the v1 trace).
  * softmax normalization deferred to AFTER the AllToAll: the A^T shards
    and their denominator rows travel fp32 straight from PSUM via DMA
    (no DVE copy, no per-fc DVE reciprocal - v1 burned 63us in
    nc.vector.reciprocal on 1 partition). Denominators are inverted
    post-A2A with reciprocal_approx_fast on 16 partitions (~1us), then
    broadcast via a tiny ones-matmul and multiplied in while converting
    to bf16.
  * AllToAll split per batch: A2A(b0) overlaps b1's projections and
    attention; only A2A(b1)'s ~9us transfer is exposed, covered by b0's
    output projection.
  * output resharding: core c produces f-rows [256c, 256c+256) of BOTH
    batches so each A2A is a balanced 8-way exchange.
  * chunked input DMAs (1MB) so projection matmuls start ~3us in.
"""

from contextlib import ExitStack

import ml_dtypes
import numpy as np

import concourse.bass as bass  # noqa: F401
import concourse.mybir as mybir
import concourse.tile as tile
from concourse import bacc
from concourse.bass_utils import run_bass_kernel_spmd

B, F, T, HID, NH, DH = 2, 2048, 2048, 1024, 16, 64
HT = HID // 128  # 8 h-tiles
TT = T // 128  # 16 key tiles
FC = F // 512  # 4 query chunks per batch
FS = 256  # f-rows per core per batch
SEG = 130 * 256  # A2A shard: 128 A^T rows + 2 denom rows, 256 f-cols
BF16, F32 = mybir.dt.bfloat16, mybir.dt.float32
I16 = mybir.dt.int16
NPBF16 = ml_dtypes.bfloat16

# Schraudolph bf16 exp: bits(int16) = z * (128/ln2) + BSCHR, z = s/8.
ASCHR = 16.0 / float(np.log(2.0))  # 128/ln2/8
BSCHR = 16250.375  # calibrated (rel err +-3.3%, robust to trunc/nearest)

OB_CONST = np.kron(np.eye(2), np.ones((1, 64))).astype(np.float32)

_CACHE: dict = {}


def _build():
    nc = bacc.Bacc("TRN2", target_bir_lowering=False, debug=False, num_devices=8)

    qT = nc.declare_dram_parameter("qT", [B, HID, F], BF16, isOutput=False)
    sT = nc.declare_dram_parameter("sT", [B, HID, T], BF16, isOutput=False)
    w3 = nc.declare_dram_parameter("w3", [HID, 384], BF16, isOutput=False)
    wo = nc.declare_dram_parameter("wo", [HID, HID], BF16, isOutput=False)
    ob = nc.declare_dram_parameter("ob", [2, 128], F32, isOutput=False)
    out = nc.declare_dram_parameter("out", [2 * FS, HID], F32, isOutput=True)

    a2a_in = [nc.dram_tensor(f"a2a_in{b}", [8, SEG], BF16) for b in range(B)]
    a2a_out = [nc.dram_tensor(f"a2a_out{b}", [8, SEG], BF16) for b in range(B)]

    with tile.TileContext(nc) as tc, ExitStack() as ctx:
        persist = ctx.enter_context(tc.tile_pool(name="persist", bufs=1))
        kT_sb = persist.tile([128, B, T], BF16, tag="kT")
        v_sb = persist.tile([128, B, TT, 2, DH + 1], BF16, tag="v")
        qTp_sb = persist.tile([128, B, F], BF16, tag="qTp")
        wo_sb = persist.tile([128, HT, HID], BF16, tag="wo")
        w3_sb = persist.tile([128, HT, 3, 128], BF16, tag="w3")  # wq|wk|wv
        onesb = persist.tile([2, 128], F32, tag="onesb")  # head-block ones

        nc.vector.memset(v_sb[:, :, :, :, DH : DH + 1], 1.0)
        nc.sync.dma_start(out=onesb[:, :], in_=ob[:, :])
        nc.sync.dma_start(
            out=w3_sb[:, :, :, :],
            in_=w3[:, :].rearrange("(a p) (k n) -> p a k n", p=128, n=128),
        )
        # wo arrives on the scalar queue; only needed post-A2A.
        nc.scalar.dma_start(
            out=wo_sb[:, :, :], in_=wo[:, :].rearrange("(a p) n -> p a n", p=128)
        )

        with (
            tc.tile_pool(name="qin", bufs=2) as qin_pool,
            tc.tile_pool(name="sin", bufs=2) as sin_pool,
            tc.tile_pool(name="ptp", bufs=6) as pt_pool,
            tc.tile_pool(name="stg", bufs=2) as stg_pool,
            tc.tile_pool(name="s_ps", bufs=2, space="PSUM") as s_ps_pool,
            tc.tile_pool(name="a_ps", bufs=1, space="PSUM") as a_ps_pool,
            tc.tile_pool(name="pj_ps", bufs=2, space="PSUM") as pj_ps,
        ):

            def proj_batch(b):
                """Q/K/V projections for batch b, chunked by 512 cols."""
                for c in range(4):
                    qc = qin_pool.tile([128, HT, 512], BF16, tag="qc")
                    nc.sync.dma_start(
                        out=qc[:, :, :],
                        in_=qT[b, :, 512 * c : 512 * (c + 1)].rearrange(
                            "(a p) n -> p a n", p=128
                        ),
                    )
                    sc = sin_pool.tile([128, HT, 512], BF16, tag="sc")
                    nc.sync.dma_start(
                        out=sc[:, :, :],
                        in_=sT[b, :, 512 * c : 512 * (c + 1)].rearrange(
                            "(a p) n -> p a n", p=128
                        ),
                    )
                    ps = pj_ps.tile([128, 512], F32, tag="pj", name="ps_q")
                    for ht in range(HT):
                        nc.tensor.matmul(
                            ps[:, :],
                            lhsT=w3_sb[:, ht, 0, :],
                            rhs=qc[:, ht, :],
                            start=(ht == 0),
                            stop=(ht == HT - 1),
                        )
                    nc.vector.tensor_copy(
                        out=qTp_sb[:, b, 512 * c : 512 * (c + 1)], in_=ps[:, :]
                    )
                    ps = pj_ps.tile([128, 512], F32, tag="pj", name="ps_k")
                    for ht in range(HT):
                        nc.tensor.matmul(
                            ps[:, :],
                            lhsT=w3_sb[:, ht, 1, :],
                            rhs=sc[:, ht, :],
                            start=(ht == 0),
                            stop=(ht == HT - 1),
                        )
                    nc.vector.tensor_copy(
                        out=kT_sb[:, b, 512 * c : 512 * (c + 1)], in_=ps[:, :]
                    )
                    for i in range(4):
                        tt = 4 * c + i
                        ps = pj_ps.tile([128, 512], F32, tag="pj", name="ps_v")[:, 0:128]
                        for ht in range(HT):
                            nc.tensor.matmul(
                                ps[:, :],
                                lhsT=sc[:, ht, 128 * i : 128 * (i + 1)],
                                rhs=w3_sb[:, ht, 2, :],
                                start=(ht == 0),
                                stop=(ht == HT - 1),
                            )
                        nc.vector.tensor_copy(
                            out=v_sb[:, b, tt, :, 0:DH],
                            in_=ps[:, :].rearrange("p (j d) -> p j d", j=2),
                        )

            def emit_s_exp(b, fc, tt):
                """S matmuls (row-tiled pair) + exp: ACT for head 0,
                Schraudolph-on-DVE for head 1."""
                sp = s_ps_pool.tile([128, 2, 512], F32, tag="s")
                for j in range(2):
                    nc.tensor.matmul(
                        sp[:, j, :],
                        lhsT=kT_sb[
                            64 * j : 64 * (j + 1), b, 128 * tt : 128 * (tt + 1)
                        ],
                        rhs=qTp_sb[
                            64 * j : 64 * (j + 1), b, 512 * fc : 512 * (fc + 1)
                        ],
                        start=True,
                        stop=True,
                        tile_position=(64 * j, 0),
                    )
                pt = pt_pool.tile([128, 2, 512], BF16, tag="pt")
                nc.scalar.activation(
                    out=pt[:, 0, :],
                    in_=sp[:, 0, :],
                    func=mybir.ActivationFunctionType.Exp,
                    scale=float(DH) ** -0.5,
                )
                with nc.allow_low_precision("schraudolph bf16 exp"):
                    nc.vector.tensor_scalar(
                        out=pt[:, 1, :].bitcast(I16),
                        in0=sp[:, 1, :],
                        scalar1=ASCHR,
                        scalar2=BSCHR,
                        op0=mybir.AluOpType.mult,
                        op1=mybir.AluOpType.add,
                    )
                return pt

            def flush(b, fc, a_ps):
                """Copy unnormalized A^T + denom rows to bf16 staging (DMA
                cannot read PSUM), then DMA into the A2A input shards for
                the 2 dest cores."""
                stf = stg_pool.tile([65, 2, 512], BF16, tag="stf")
                with nc.allow_low_precision("bf16 a2a shard"):
                    nc.vector.tensor_copy(out=stf[:, :, :], in_=a_ps[:, :, :])
                for h in range(2):
                    dst = 2 * fc + h
                    sl = np.s_[256 * h : 256 * (h + 1)]
                    nc.scalar.dma_start(
                        out=a2a_in[b][dst, 0:32768].rearrange(
                            "(j p n) -> p j n", j=2, p=64, n=256
                        ),
                        in_=stf[0:64, :, sl],
                    )
                    nc.scalar.dma_start(
                        out=a2a_in[b][dst, 32768:33280].rearrange(
                            "(o j n) -> o j n", o=1, j=2, n=256
                        ),
                        in_=stf[64:65, :, sl],
                    )

            def attention_batch(b):
                steps = [(fc, tt) for fc in range(FC) for tt in range(TT)]
                pts = {}
                pts[steps[0]] = emit_s_exp(b, *steps[0])
                a_tiles = {}
                for i, (fc, tt) in enumerate(steps):
                    if tt == 0:
                        a_tiles[fc] = a_ps_pool.tile(
                            [65, 2, 512], F32, tag="a", name="a_acc"
                        )
                    if i + 1 < len(steps):
                        pts[steps[i + 1]] = emit_s_exp(b, *steps[i + 1])
                    a_ps = a_tiles[fc]
                    pt = pts.pop((fc, tt))
                    for j in range(2):
                        nc.tensor.matmul(
                            a_ps[:, j, :],
                            lhsT=v_sb[:, b, tt, j, :],
                            rhs=pt[:, j, :],
                            start=(tt == 0),
                            stop=(tt == TT - 1),
                        )
                    if tt == TT - 1:
                        flush(b, fc, a_ps)
                        del a_tiles[fc]

            proj_batch(0)
            attention_batch(0)
            nc.gpsimd.collective_compute(
                "AllToAll",
                mybir.AluOpType.bypass,
                replica_groups=[[0, 1, 2, 3, 4, 5, 6, 7]],
                ins=[a2a_in[0].ap().opt()],
                outs=[a2a_out[0].ap().opt()],
            )
            proj_batch(1)
            attention_batch(1)
            nc.gpsimd.collective_compute(
                "AllToAll",
                mybir.AluOpType.bypass,
                replica_groups=[[0, 1, 2, 3, 4, 5, 6, 7]],
                ins=[a2a_in[1].ap().opt()],
                outs=[a2a_out[1].ap().opt()],
            )

        # ---- post-A2A: normalize + output projection, per batch --------
        with (
            tc.tile_pool(name="atg", bufs=2) as atg_pool,
            tc.tile_pool(name="den", bufs=2) as den_pool,
            tc.tile_pool(name="stp", bufs=16) as st_pool,
            tc.tile_pool(name="bc_ps", bufs=3, space="PSUM") as bc_ps_pool,
            tc.tile_pool(name="o_ps", bufs=2, space="PSUM") as o_ps_pool,
        ):
            for b in range(B):
                atg = atg_pool.tile([128, 8, 256], BF16, tag="atg")
                nc.scalar.dma_start(
                    out=atg[:, :, :],
                    in_=a2a_out[b][:, 0:32768].rearrange(
                        "a (r n) -> r a n", r=128, n=256
                    ),
                )
                den16 = den_pool.tile([16, 256], BF16, tag="den16")
                for a in range(8):
                    nc.scalar.dma_start(
                        out=den16[2 * a : 2 * a + 2, :],
                        in_=a2a_out[b][a, 32768:33280].rearrange(
                            "(j n) -> j n", j=2, n=256
                        ),
                    )
                den16f = den_pool.tile([16, 256], F32, tag="den16f")
                nc.vector.tensor_copy(out=den16f[:, :], in_=den16[:, :])
                den16r = den_pool.tile([16, 256], F32, tag="den16r")
                nc.vector.reciprocal_approx_fast(
                    out=den16r[:, :], in_=den16f[:, :]
                )
                # [16,256] -> [2, 8, 256] so each chunk's denom pair sits at
                # partitions 0-1 for the broadcast matmul rhs.
                den2 = den_pool.tile([2, 8, 256], F32, tag="den2")
                for a in range(8):
                    nc.scalar.dma_start(
                        out=den2[:, a, :], in_=den16r[2 * a : 2 * a + 2, :]
                    )
                sts = []
                for a in range(8):
                    bc = bc_ps_pool.tile([128, 256], F32, tag="bc")
                    nc.tensor.matmul(
                        bc[:, :],
                        lhsT=onesb[:, :],
                        rhs=den2[:, a, :],
                        start=True,
                        stop=True,
                    )
                    st = st_pool.tile([128, 256], BF16, tag="st")
                    nc.vector.tensor_mul(
                        out=st[:, :], in0=atg[:, a, :], in1=bc[:, :]
                    )
                    sts.append(st)
                for ft in range(2):
                    o_ps = o_ps_pool.tile([128, 2, 512], F32, tag="o")
                    for a in range(8):
                        for jj in range(2):
                            nc.tensor.matmul(
                                o_ps[:, jj, :],
                                lhsT=sts[a][:, 128 * ft : 128 * (ft + 1)],
                                rhs=wo_sb[:, a, 512 * jj : 512 * (jj + 1)],
                                start=(a == 0),
                                stop=(a == 7),
                            )
                    ot = st_pool.tile([128, HID], F32, tag="ot")
                    eng = nc.vector if ft == 0 else nc.scalar
                    if ft == 0:
                        nc.vector.tensor_copy(
                            out=ot[:, :].rearrange("p (j n) -> p j n", j=2),
                            in_=o_ps[:, :, :],
                        )
                    else:
                        nc.scalar.copy(
                            out=ot[:, :].rearrange("p (j n) -> p j n", j=2),
                            in_=o_ps[:, :, :],
                        )
                    nc.sync.dma_start(
                        out=out[256 * b + 128 * ft : 256 * b + 128 * (ft + 1), :],
                        in_=ot[:, :],
                    )

    nc.compile()
    return nc


def _get_nc():
    if "nc" not in _CACHE:
        _CACHE["nc"] = _build()
    return _CACHE["nc"]


def _reference_fallback(query_input, source_input, bias, wq, wk, wv, wo):
    """Numpy fallback, only used if bias is unexpectedly nonzero."""
    q = np.einsum("bfh,hnd->bfnd", query_input, wq) * (DH**-0.5)
    k = np.einsum("bth,hnd->btnd", source_input, wk)
    v = np.einsum("bth,hnd->btnd", source_input, wv)
    logits = np.einsum("btnd,bfnd->bnft", k, q) + bias
    logits -= logits.max(axis=-1, keepdims=True)
    w = np.exp(logits)
    w /= w.sum(axis=-1, keepdims=True)
    attn = np.einsum("bnft,btnd->bfnd", w, v)
    return np.einsum("bfnd,ndh->bfh", attn, wo).astype(np.float32)


def make_in_maps(query_input, source_input, wq, wk, wv, wo):
    wo2 = np.ascontiguousarray(wo.reshape(HID, HID).astype(NPBF16))
    qTb = np.ascontiguousarray(
        np.transpose(query_input, (0, 2, 1))
    ).astype(NPBF16)  # [B, HID, F]
    sTb = np.ascontiguousarray(np.transpose(source_input, (0, 2, 1))).astype(NPBF16)
    wqh = wq.reshape(HID, NH, DH)
    wkh = wk.reshape(HID, NH, DH)
    wvh = wv.reshape(HID, NH, DH)

    in_maps = []
    for c in range(8):
        sl = np.s_[:, 2 * c : 2 * c + 2, :]
        w3c = np.concatenate(
            [
                wqh[sl].reshape(HID, 128),
                wkh[sl].reshape(HID, 128),
                wvh[sl].reshape(HID, 128),
            ],
            axis=1,
        )
        in_maps.append(
            {
                "qT": qTb,
                "sT": sTb,
                "w3": np.ascontiguousarray(w3c).astype(NPBF16),
                "wo": wo2,
                "ob": OB_CONST,
            }
        )
    return in_maps


def assemble(results):
    """results[c]["out"] is [512, 1024]: rows 0-255 = batch 0 f-slice
    [256c, 256c+256), rows 256-511 = batch 1 same slice."""
    out_full = np.empty((B, F, HID), dtype=np.float32)
    for c in range(8):
        r = results[c]["out"]
        out_full[0, FS * c : FS * (c + 1), :] = r[0:FS]
        out_full[1, FS * c : FS * (c + 1), :] = r[FS : 2 * FS]
    return out_full


def kernel(query_input, source_input, bias, wq, wk, wv, wo):
    query_input = np.asarray(query_input, dtype=np.float32)
    source_input = np.asarray(source_input, dtype=np.float32)
    bias = np.asarray(bias, dtype=np.float32)
    wq = np.asarray(wq, dtype=np.float32)
    wk = np.asarray(wk, dtype=np.float32)
    wv = np.asarray(wv, dtype=np.float32)
    wo = np.asarray(wo, dtype=np.float32)

    if np.any(bias):
        return _reference_fallback(query_input, source_input, bias, wq, wk, wv, wo)

    in_maps = make_in_maps(query_input, source_input, wq, wk, wv, wo)
    nc = _get_nc()
    res = run_bass_kernel_spmd(nc, in_maps, core_ids=list(range(8)))
    return assemble(res.results)


# revision 12
# speedup vs baseline: 1.4131x; 1.3136x over previous
"""Multi-head attention (B=2, F=T=2048, H=1024, 16 heads x 64) on 8 TRN2
NeuronCores.

v3 design:
  * exp split across engines: per attention step, head j=0's exp runs on
    ACT (exact, free affine scale), head j=1's exp runs on DVE as a
    one-instruction Schraudolph bf16 exp (tensor_scalar mult+add with
    int16 output whose bits ARE the bf16 exp).
  * softmax normalization deferred to AFTER the AllToAll: shards carry 2
    denominator rows; denominators are inverted post-A2A with
    reciprocal_approx_fast on 16 partitions and broadcast via a tiny
    bf16 ones-matmul, folded into the bf16 conversion multiply.
  * AllToAll split per batch; A2A(b0) overlaps b1's attention. Output
    resharding: core c produces f-rows [256c, 256c+256) of BOTH batches.
  * work interleaving: batch-1 projections are emitted as side-closures
    inside batch-0's attention step loop (keeps PE dense/warm and the
    exp engines fed); batch-0's output projection is interleaved into
    batch-1's attention. Only b1's A2A + output projection are exposed.
  * inputs arrive chunk-major ([B*4, 128, HT*512]) so every DMA line is
    8KB contiguous.
"""

from contextlib import ExitStack

import ml_dtypes
import numpy as np

import concourse.bass as bass  # noqa: F401
import concourse.mybir as mybir
import concourse.tile as tile
from concourse import bacc
from concourse.bass_utils import run_bass_kernel_spmd

B, F, T, HID, NH, DH = 2, 2048, 2048, 1024, 16, 64
HT = HID // 128  # 8 h-tiles
TT = T // 128  # 16 key tiles
FC = F // 512  # 4 query chunks per batch
FS = 256  # f-rows per core per batch
SEG = 130 * 256  # A2A shard: 128 A^T rows + 2 denom rows, 256 f-cols
BF16, F32 = mybir.dt.bfloat16, mybir.dt.float32
I16 = mybir.dt.int16
NPBF16 = ml_dtypes.bfloat16

# Schraudolph bf16 exp: bits(int16) = s * ASCHR + BSCHR, s = raw logits.
ASCHR = 16.0 / float(np.log(2.0))  # (128/ln2) * (1/8 logit scale)
BSCHR = 16250.375  # calibrated; rel err +-3.3%, robust to trunc/nearest

OB_CONST = np.kron(np.eye(2), np.ones((1, 64))).astype(NPBF16)

_CACHE: dict = {}


def _build():
    nc = bacc.Bacc("TRN2", target_bir_lowering=False, debug=False, num_devices=8)

    qT = nc.declare_dram_parameter("qT", [B * 4, 128, HT * 512], BF16, isOutput=False)
    sT = nc.declare_dram_parameter("sT", [B * 4, 128, HT * 512], BF16, isOutput=False)
    w3 = nc.declare_dram_parameter("w3", [HID, 384], BF16, isOutput=False)
    wo = nc.declare_dram_parameter("wo", [HID, HID], BF16, isOutput=False)
    ob = nc.declare_dram_parameter("ob", [2, 128], BF16, isOutput=False)
    out = nc.declare_dram_parameter("out", [2 * FS, HID], F32, isOutput=True)

    a2a_in = [nc.dram_tensor(f"a2a_in{b}", [8, SEG], BF16) for b in range(B)]
    a2a_out = [nc.dram_tensor(f"a2a_out{b}", [8, SEG], BF16) for b in range(B)]

    with tile.TileContext(nc) as tc, ExitStack() as ctx:
        persist = ctx.enter_context(tc.tile_pool(name="persist", bufs=1))
        kT_sb = persist.tile([128, B, T], BF16, tag="kT")
        v_sb = persist.tile([128, B, TT, 2, DH + 1], BF16, tag="v")
        qTp_sb = persist.tile([128, B, F], BF16, tag="qTp")
        wo_sb = persist.tile([128, HT, HID], BF16, tag="wo")
        w3_sb = persist.tile([128, HT, 3, 128], BF16, tag="w3")  # wq|wk|wv
        onesb = persist.tile([2, 128], BF16, tag="onesb")

        nc.vector.memset(v_sb[:, :, :, :, DH : DH + 1], 1.0)
        nc.sync.dma_start(out=onesb[:, :], in_=ob[:, :])
        nc.sync.dma_start(
            out=w3_sb[:, :, :, :],
            in_=w3[:, :].rearrange("(a p) (k n) -> p a k n", p=128, n=128),
        )
        nc.scalar.dma_start(
            out=wo_sb[:, :, :], in_=wo[:, :].rearrange("(a p) n -> p a n", p=128)
        )

        with (
            tc.tile_pool(name="qin", bufs=3) as qin_pool,
            tc.tile_pool(name="sin", bufs=3) as sin_pool,
            tc.tile_pool(name="ptp", bufs=6) as pt_pool,
            tc.tile_pool(name="stg", bufs=3) as stg_pool,
            tc.tile_pool(name="den", bufs=2) as den_pool,
            tc.tile_pool(name="atg", bufs=2) as atg_pool,
            tc.tile_pool(name="stp", bufs=12) as st_pool,
            tc.tile_pool(name="otp", bufs=4) as ot_pool,
            tc.tile_pool(name="s_ps", bufs=2, space="PSUM") as s_ps_pool,
            tc.tile_pool(name="a_ps", bufs=1, space="PSUM") as a_ps_pool,
            tc.tile_pool(name="pj_ps", bufs=2, space="PSUM") as pj_ps,
        ):

            def input_dmas(b):
                """Issue all chunk DMAs for batch b; returns tiles."""
                qs, ss = [], []
                for c in range(4):
                    qc = qin_pool.tile([128, HT, 512], BF16, tag="qc")
                    nc.sync.dma_start(
                        out=qc[:, :, :],
                        in_=qT[4 * b + c, :, :].rearrange("p (a n) -> p a n", a=HT),
                    )
                    sc = sin_pool.tile([128, HT, 512], BF16, tag="sc")
                    nc.sync.dma_start(
                        out=sc[:, :, :],
                        in_=sT[4 * b + c, :, :].rearrange("p (a n) -> p a n", a=HT),
                    )
                    qs.append(qc)
                    ss.append(sc)
                return qs, ss

            def proj_closures(b, qs, ss, use_scalar_copies):
                """Per-chunk projection closures (each emits PE matmuls and
                one PSUM->SBUF copy)."""
                cls = []

                def qk_copy(dst, src):
                    if use_scalar_copies:
                        nc.scalar.copy(out=dst, in_=src)
                    else:
                        nc.vector.tensor_copy(out=dst, in_=src)

                def mk_k(c):
                    def f():
                        ps = pj_ps.tile([128, 512], F32, tag="pj", name="ps_k")
                        for ht in range(HT):
                            nc.tensor.matmul(
                                ps[:, :],
                                lhsT=w3_sb[:, ht, 1, :],
                                rhs=ss[c][:, ht, :],
                                start=(ht == 0),
                                stop=(ht == HT - 1),
                            )
                        qk_copy(kT_sb[:, b, 512 * c : 512 * (c + 1)], ps[:, :])
                    return f

                def mk_v(c, i):
                    def f():
                        tt = 4 * c + i
                        ps = pj_ps.tile([128, 512], F32, tag="pj", name="ps_v")
                        for ht in range(HT):
                            nc.tensor.matmul(
                                ps[:, 0:128],
                                lhsT=ss[c][:, ht, 128 * i : 128 * (i + 1)],
                                rhs=w3_sb[:, ht, 2, :],
                                start=(ht == 0),
                                stop=(ht == HT - 1),
                            )
                        nc.vector.tensor_copy(
                            out=v_sb[:, b, tt, :, 0:DH],
                            in_=ps[:, 0:128].rearrange("p (j d) -> p j d", j=2),
                        )
                    return f

                def mk_q(c):
                    def f():
                        ps = pj_ps.tile([128, 512], F32, tag="pj", name="ps_q")
                        for ht in range(HT):
                            nc.tensor.matmul(
                                ps[:, :],
                                lhsT=w3_sb[:, ht, 0, :],
                                rhs=qs[c][:, ht, :],
                                start=(ht == 0),
                                stop=(ht == HT - 1),
                            )
                        qk_copy(qTp_sb[:, b, 512 * c : 512 * (c + 1)], ps[:, :])
                    return f

                for c in range(4):
                    cls.append(mk_k(c))
                    for i in range(4):
                        cls.append(mk_v(c, i))
                    cls.append(mk_q(c))
                return cls  # 24 closures (6 per chunk), PE-ordered K,V...,Q

            def emit_s_exp(b, fc, tt):
                sp = s_ps_pool.tile([128, 2, 512], F32, tag="s")
                for j in range(2):
                    nc.tensor.matmul(
                        sp[:, j, :],
                        lhsT=kT_sb[
                            64 * j : 64 * (j + 1), b, 128 * tt : 128 * (tt + 1)
                        ],
                        rhs=qTp_sb[
                            64 * j : 64 * (j + 1), b, 512 * fc : 512 * (fc + 1)
                        ],
                        start=True,
                        stop=True,
                        tile_position=(64 * j, 0),
                    )
                pt = pt_pool.tile([128, 2, 512], BF16, tag="pt")
                nc.scalar.activation(
                    out=pt[:, 0, :],
                    in_=sp[:, 0, :],
                    func=mybir.ActivationFunctionType.Exp,
                    scale=float(DH) ** -0.5,
                )
                with nc.allow_low_precision("schraudolph bf16 exp"):
                    nc.vector.tensor_scalar(
                        out=pt[:, 1, :].bitcast(I16),
                        in0=sp[:, 1, :],
                        scalar1=ASCHR,
                        scalar2=BSCHR,
                        op0=mybir.AluOpType.mult,
                        op1=mybir.AluOpType.add,
                    )
                return pt

            def flush(b, fc, a_ps):
                stf = stg_pool.tile([65, 2, 512], BF16, tag="stf")
                with nc.allow_low_precision("bf16 a2a shard"):
                    if fc % 2 == 0:
                        nc.vector.tensor_copy(out=stf[:, :, :], in_=a_ps[:, :, :])
                    else:
                        nc.scalar.copy(out=stf[:, :, :], in_=a_ps[:, :, :])
                for h in range(2):
                    dst = 2 * fc + h
                    sl = np.s_[256 * h : 256 * (h + 1)]
                    nc.scalar.dma_start(
                        out=a2a_in[b][dst, 0:32768].rearrange(
                            "(j p n) -> p j n", j=2, p=64, n=256
                        ),
                        in_=stf[0:64, :, sl],
                    )
                    nc.scalar.dma_start(
                        out=a2a_in[b][dst, 32768:33280].rearrange(
                            "(o j n) -> o j n", o=1, j=2, n=256
                        ),
                        in_=stf[64:65, :, sl],
                    )

            def outproj_closures(b):
                """Post-A2A(b): reciprocal + broadcast + normalize-multiply +
                output projection, as closures."""
                cls = []
                den16 = den_pool.tile([16, 256], BF16, tag="den16")
                den16f = den_pool.tile([16, 256], F32, tag="den16f")
                den16r = den_pool.tile([16, 256], F32, tag="den16r")
                den16rb = den_pool.tile([16, 256], BF16, tag="den16rb")
                den2 = den_pool.tile([2, 8, 256], BF16, tag="den2")
                atg = atg_pool.tile([128, 8, 256], BF16, tag="atg")

                def gather():
                    for a in range(8):
                        nc.sync.dma_start(
                            out=den16[2 * a : 2 * a + 2, :],
                            in_=a2a_out[b][a, 32768:33280].rearrange(
                                "(j n) -> j n", j=2, n=256
                            ),
                        )
                    nc.scalar.dma_start(
                        out=atg[:, :, :],
                        in_=a2a_out[b][:, 0:32768].rearrange(
                            "a (r n) -> r a n", r=128, n=256
                        ),
                    )

                def recip():
                    nc.vector.tensor_copy(out=den16f[:, :], in_=den16[:, :])
                    nc.vector.reciprocal_approx_fast(
                        out=den16r[:, :], in_=den16f[:, :]
                    )
                    with nc.allow_low_precision("bf16 denom recip"):
                        nc.vector.tensor_copy(out=den16rb[:, :], in_=den16r[:, :])
                    for a in range(8):
                        nc.sync.dma_start(
                            out=den2[:, a, :], in_=den16rb[2 * a : 2 * a + 2, :]
                        )

                cls.append(gather)
                cls.append(recip)
                sts = [None] * 8

                def mk_bc(a):
                    def f():
                        bc = pj_ps.tile([128, 512], F32, tag="pj", name="bc")
                        nc.tensor.matmul(
                            bc[:, 0:256],
                            lhsT=onesb[:, :],
                            rhs=den2[:, a, :],
                            start=True,
                            stop=True,
                        )
                        st = st_pool.tile([128, 256], BF16, tag="st")
                        with nc.allow_low_precision("bf16 attn normalize"):
                            nc.vector.tensor_mul(
                                out=st[:, :], in0=atg[:, a, :], in1=bc[:, 0:256]
                            )
                        sts[a] = st
                    return f

                for a in range(8):
                    cls.append(mk_bc(a))

                def mk_o(ft, jj):
                    def f():
                        o_ps = pj_ps.tile([128, 512], F32, tag="pj", name="o")
                        for a in range(8):
                            nc.tensor.matmul(
                                o_ps[:, :],
                                lhsT=sts[a][:, 128 * ft : 128 * (ft + 1)],
                                rhs=wo_sb[:, a, 512 * jj : 512 * (jj + 1)],
                                start=(a == 0),
                                stop=(a == 7),
                            )
                        ot = ot_pool.tile([128, 512], F32, tag="ot")
                        if jj == 0:
                            nc.vector.tensor_copy(out=ot[:, :], in_=o_ps[:, :])
                        else:
                            nc.scalar.copy(out=ot[:, :], in_=o_ps[:, :])
                        nc.scalar.dma_start(
                            out=out[
                                256 * b + 128 * ft : 256 * b + 128 * (ft + 1),
                                512 * jj : 512 * (jj + 1),
                            ],
                            in_=ot[:, :],
                        )
                    return f

                for ft in range(2):
                    for jj in range(2):
                        cls.append(mk_o(ft, jj))
                return cls  # 14 closures

            def attention_batch(b, sched):
                """sched: dict step -> list of closures to emit at that step."""
                steps = [(fc, tt) for fc in range(FC) for tt in range(TT)]
                pts = {}
                pts[steps[0]] = emit_s_exp(b, *steps[0])
                a_tiles = {}
                for i, (fc, tt) in enumerate(steps):
                    for cl in sched.get(i, ()):
                        cl()
                    if tt == 0:
                        a_tiles[fc] = a_ps_pool.tile(
                            [65, 2, 512], F32, tag="a", name="a_acc"
                        )
                    if i + 1 < len(steps):
                        pts[steps[i + 1]] = emit_s_exp(b, *steps[i + 1])
                    a_ps = a_tiles[fc]
                    pt = pts.pop((fc, tt))
                    for j in range(2):
                        nc.tensor.matmul(
                            a_ps[:, j, :],
                            lhsT=v_sb[:, b, tt, j, :],
                            rhs=pt[:, j, :],
                            start=(tt == 0),
                            stop=(tt == TT - 1),
                        )
                    if tt == TT - 1:
                        flush(b, fc, a_ps)
                        del a_tiles[fc]
                for i in sorted(k for k in sched if k >= len(steps)):
                    for cl in sched[i]:
                        cl()

            def spread(closures, start, end):
                """Schedule closures evenly over steps [start, end)."""
                sched = {}
                n = len(closures)
                for idx, cl in enumerate(closures):
                    step = start + idx * (end - start) // n
                    sched.setdefault(step, []).append(cl)
                return sched

            # ---- phase structure ------------------------------------------
            q0, s0 = input_dmas(0)
            p0 = proj_closures(0, q0, s0, use_scalar_copies=False)
            # chunk 0 of batch 0 runs serially (ramp)
            for cl in p0[0:6]:
                cl()
            q1, s1 = input_dmas(1)
            p1 = proj_closures(1, q1, s1, use_scalar_copies=True)

            # b0 attention: b0 chunks 1-3 early (chunk c needed by step 4c),
            # then b1's projections spread over the rest.
            sched0 = {}
            for c in range(1, 4):
                base = p0[6 * c : 6 * c + 6]
                for idx, cl in enumerate(base):
                    sched0.setdefault(4 * (c - 1) + idx // 3, []).append(cl)
            for step, cls in spread(p1, 14, 62).items():
                sched0.setdefault(step, []).extend(cls)
            attention_batch(0, sched0)

            nc.gpsimd.collective_compute(
                "AllToAll",
                mybir.AluOpType.bypass,
                replica_groups=[[0, 1, 2, 3, 4, 5, 6, 7]],
                ins=[a2a_in[0].ap().opt()],
                outs=[a2a_out[0].ap().opt()],
            )

            # b1 attention with b0's output projection interleaved late.
            op0 = outproj_closures(0)
            sched1 = spread(op0, 30, 62)
            attention_batch(1, sched1)

            nc.gpsimd.collective_compute(
                "AllToAll",
                mybir.AluOpType.bypass,
                replica_groups=[[0, 1, 2, 3, 4, 5, 6, 7]],
                ins=[a2a_in[1].ap().opt()],
                outs=[a2a_out[1].ap().opt()],
            )

            for cl in outproj_closures(1):
                cl()

    nc.compile()
    return nc


def _get_nc():
    if "nc" not in _CACHE:
        _CACHE["nc"] = _build()
    return _CACHE["nc"]


def _reference_fallback(query_input, source_input, bias, wq, wk, wv, wo):
    """Numpy fallback, only used if bias is unexpectedly nonzero."""
    q = np.einsum("bfh,hnd->bfnd", query_input, wq) * (DH**-0.5)
    k = np.einsum("bth,hnd->btnd", source_input, wk)
    v = np.einsum("bth,hnd->btnd", source_input, wv)
    logits = np.einsum("btnd,bfnd->bnft", k, q) + bias
    logits -= logits.max(axis=-1, keepdims=True)
    w = np.exp(logits)
    w /= w.sum(axis=-1, keepdims=True)
    attn = np.einsum("bnft,btnd->bfnd", w, v)
    return np.einsum("bfnd,ndh->bfh", attn, wo).astype(np.float32)


def _chunk_major(xT):
    """[B, HID, F] -> [B*4, 128, HT*512] with 8KB-contiguous lines:
    out[4b+c, p, a*512+n] = xT[b, a*128+p, 512c+n]."""
    o = np.empty((B * 4, 128, HT * 512), dtype=NPBF16)
    for b in range(B):
        for c in range(4):
            chunk = xT[b, :, 512 * c : 512 * (c + 1)]  # [1024, 512]
            o[4 * b + c] = (
                chunk.reshape(HT, 128, 512).transpose(1, 0, 2).reshape(128, HT * 512)
            )
    return np.ascontiguousarray(o)


def make_in_maps(query_input, source_input, wq, wk, wv, wo):
    wo2 = np.ascontiguousarray(wo.reshape(HID, HID).astype(NPBF16))
    qTb = _chunk_major(np.transpose(query_input, (0, 2, 1)).astype(NPBF16))
    sTb = _chunk_major(np.transpose(source_input, (0, 2, 1)).astype(NPBF16))
    wqh = wq.reshape(HID, NH, DH)
    wkh = wk.reshape(HID, NH, DH)
    wvh = wv.reshape(HID, NH, DH)

    in_maps = []
    for c in range(8):
        sl = np.s_[:, 2 * c : 2 * c + 2, :]
        w3c = np.concatenate(
            [
                wqh[sl].reshape(HID, 128),
                wkh[sl].reshape(HID, 128),
                wvh[sl].reshape(HID, 128),
            ],
            axis=1,
        )
        in_maps.append(
            {
                "qT": qTb,
                "sT": sTb,
                "w3": np.ascontiguousarray(w3c).astype(NPBF16),
                "wo": wo2,
                "ob": OB_CONST,
            }
        )
    return in_maps


def assemble(results):
    """results[c]["out"] is [512, 1024]: rows 0-255 = batch 0 f-slice
    [256c, 256c+256), rows 256-511 = batch 1 same slice."""
    out_full = np.empty((B, F, HID), dtype=np.float32)
    for c in range(8):
        r = results[c]["out"]
        out_full[0, FS * c : FS * (c + 1), :] = r[0:FS]
        out_full[1, FS * c : FS * (c + 1), :] = r[FS : 2 * FS]
    return out_full


def kernel(query_input, source_input, bias, wq, wk, wv, wo):
    query_input = np.asarray(query_input, dtype=np.float32)
    source_input = np.asarray(source_input, dtype=np.float32)
    bias = np.asarray(bias, dtype=np.float32)
    wq = np.asarray(wq, dtype=np.float32)
    wk = np.asarray(wk, dtype=np.float32)
    wv = np.asarray(wv, dtype=np.float32)
    wo = np.asarray(wo, dtype=np.float32)

    if np.any(bias):
        return _reference_fallback(query_input, source_input, bias, wq, wk, wv, wo)

    in_maps = make_in_maps(query_input, source_input, wq, wk, wv, wo)
    nc = _get_nc()
    res = run_bass_kernel_spmd(nc, in_maps, core_ids=list(range(8)))
    return assemble(res.results)


# revision 14
# speedup vs baseline: 1.4884x; 1.0532x over previous
"""Multi-head attention (B=2, F=T=2048, H=1024, 16 heads x 64) on 8 TRN2
NeuronCores.

v3 design:
  * exp split across engines: per attention step, head j=0's exp runs on
    ACT (exact, free affine scale), head j=1's exp runs on DVE as a
    one-instruction Schraudolph bf16 exp (tensor_scalar mult+add with
    int16 output whose bits ARE the bf16 exp).
  * softmax normalization deferred to AFTER the AllToAll: shards carry 2
    denominator rows; denominators are inverted post-A2A with
    reciprocal_approx_fast on 16 partitions and broadcast via a tiny
    bf16 ones-matmul, folded into the bf16 conversion multiply.
  * AllToAll split per batch; A2A(b0) overlaps b1's attention. Output
    resharding: core c produces f-rows [256c, 256c+256) of BOTH batches.
  * work interleaving: batch-1 projections are emitted as side-closures
    inside batch-0's attention step loop (keeps PE dense/warm and the
    exp engines fed); batch-0's output projection is interleaved into
    batch-1's attention. Only b1's A2A + output projection are exposed.
  * inputs arrive chunk-major ([B*4, 128, HT*512]) so every DMA line is
    8KB contiguous.
"""

from contextlib import ExitStack

import ml_dtypes
import numpy as np

import concourse.bass as bass  # noqa: F401
import concourse.mybir as mybir
import concourse.tile as tile
from concourse import bacc
from concourse.bass_utils import run_bass_kernel_spmd

B, F, T, HID, NH, DH = 2, 2048, 2048, 1024, 16, 64
HT = HID // 128  # 8 h-tiles
TT = T // 128  # 16 key tiles
FC = F // 512  # 4 query chunks per batch
FS = 256  # f-rows per core per batch
SEG = 130 * 256  # A2A shard: 128 A^T rows + 2 denom rows, 256 f-cols
BF16, F32 = mybir.dt.bfloat16, mybir.dt.float32
I16 = mybir.dt.int16
NPBF16 = ml_dtypes.bfloat16

# Schraudolph bf16 exp: bits(int16) = s * ASCHR + BSCHR, s = raw logits.
ASCHR = 16.0 / float(np.log(2.0))  # (128/ln2) * (1/8 logit scale)
BSCHR = 16250.375  # calibrated; rel err +-3.3%, robust to trunc/nearest

OB_CONST = np.kron(np.eye(2), np.ones((1, 64))).astype(NPBF16)

_CACHE: dict = {}


def _build():
    nc = bacc.Bacc("TRN2", target_bir_lowering=False, debug=False, num_devices=8)

    qT = nc.declare_dram_parameter("qT", [B * 4, 128, HT * 512], BF16, isOutput=False)
    sT = nc.declare_dram_parameter("sT", [B * 4, 128, HT * 512], BF16, isOutput=False)
    w3 = nc.declare_dram_parameter("w3", [128, 3072], BF16, isOutput=False)
    wo = nc.declare_dram_parameter("wo", [HID, HID], BF16, isOutput=False)
    ob = nc.declare_dram_parameter("ob", [2, 128], BF16, isOutput=False)
    out = nc.declare_dram_parameter("out", [2 * FS, HID], F32, isOutput=True)

    a2a_in = [nc.dram_tensor(f"a2a_in{b}", [8, SEG], BF16) for b in range(B)]
    a2a_out = [nc.dram_tensor(f"a2a_out{b}", [8, SEG], BF16) for b in range(B)]

    with tile.TileContext(nc) as tc, ExitStack() as ctx:
        persist = ctx.enter_context(tc.tile_pool(name="persist", bufs=1))
        kT_sb = persist.tile([128, B, T], BF16, tag="kT")
        v_sb = persist.tile([128, B, TT, 2, DH + 1], BF16, tag="v")
        qTp_sb = persist.tile([128, B, F], BF16, tag="qTp")
        wo_sb = persist.tile([128, HT, HID], BF16, tag="wo")
        w3_sb = persist.tile([128, HT, 3, 128], BF16, tag="w3")  # wq|wk|wv
        onesb = persist.tile([2, 128], BF16, tag="onesb")

        nc.vector.memset(v_sb[:, :, :, :, DH : DH + 1], 1.0)
        nc.sync.dma_start(out=onesb[:, :], in_=ob[:, :])
        nc.sync.dma_start(
            out=w3_sb[:, :, :, :],
            in_=w3[:, :].rearrange("p (a k n) -> p a k n", a=HT, k=3),
        )

        with (
            tc.tile_pool(name="qin", bufs=3) as qin_pool,
            tc.tile_pool(name="sin", bufs=3) as sin_pool,
            tc.tile_pool(name="ptp", bufs=6) as pt_pool,
            tc.tile_pool(name="stg", bufs=3) as stg_pool,
            tc.tile_pool(name="den", bufs=2) as den_pool,
            tc.tile_pool(name="atg", bufs=2) as atg_pool,
            tc.tile_pool(name="stp", bufs=12) as st_pool,
            tc.tile_pool(name="otp", bufs=4) as ot_pool,
            tc.tile_pool(name="s_ps", bufs=2, space="PSUM") as s_ps_pool,
            tc.tile_pool(name="a_ps", bufs=1, space="PSUM") as a_ps_pool,
            tc.tile_pool(name="pj_ps", bufs=2, space="PSUM") as pj_ps,
        ):

            def input_dmas(b):
                """Issue all chunk DMAs for batch b; returns tiles."""
                qs, ss = [], []
                for c in range(4):
                    qc = qin_pool.tile([128, HT, 512], BF16, tag="qc")
                    nc.sync.dma_start(
                        out=qc[:, :, :],
                        in_=qT[4 * b + c, :, :].rearrange("p (a n) -> p a n", a=HT),
                    )
                    sc = sin_pool.tile([128, HT, 512], BF16, tag="sc")
                    nc.sync.dma_start(
                        out=sc[:, :, :],
                        in_=sT[4 * b + c, :, :].rearrange("p (a n) -> p a n", a=HT),
                    )
                    qs.append(qc)
                    ss.append(sc)
                return qs, ss

            def proj_closures(b, qs, ss, use_scalar_copies):
                """Per-chunk projection closures (each emits PE matmuls and
                one PSUM->SBUF copy)."""
                cls = []

                def qk_copy(dst, src):
                    if use_scalar_copies:
                        nc.scalar.copy(out=dst, in_=src)
                    else:
                        nc.vector.tensor_copy(out=dst, in_=src)

                def mk_k(c):
                    def f():
                        ps = pj_ps.tile([128, 512], F32, tag="pj", name="ps_k")
                        for ht in range(HT):
                            nc.tensor.matmul(
                                ps[:, :],
                                lhsT=w3_sb[:, ht, 1, :],
                                rhs=ss[c][:, ht, :],
                                start=(ht == 0),
                                stop=(ht == HT - 1),
                            )
                        qk_copy(kT_sb[:, b, 512 * c : 512 * (c + 1)], ps[:, :])
                    return f

                def mk_v(c, i):
                    def f():
                        tt = 4 * c + i
                        ps = pj_ps.tile([128, 512], F32, tag="pj", name="ps_v")
                        for ht in range(HT):
                            nc.tensor.matmul(
                                ps[:, 0:128],
                                lhsT=ss[c][:, ht, 128 * i : 128 * (i + 1)],
                                rhs=w3_sb[:, ht, 2, :],
                                start=(ht == 0),
                                stop=(ht == HT - 1),
                            )
                        nc.vector.tensor_copy(
                            out=v_sb[:, b, tt, :, 0:DH],
                            in_=ps[:, 0:128].rearrange("p (j d) -> p j d", j=2),
                        )
                    return f

                def mk_q(c):
                    def f():
                        ps = pj_ps.tile([128, 512], F32, tag="pj", name="ps_q")
                        for ht in range(HT):
                            nc.tensor.matmul(
                                ps[:, :],
                                lhsT=w3_sb[:, ht, 0, :],
                                rhs=qs[c][:, ht, :],
                                start=(ht == 0),
                                stop=(ht == HT - 1),
                            )
                        qk_copy(qTp_sb[:, b, 512 * c : 512 * (c + 1)], ps[:, :])
                    return f

                for c in range(4):
                    cls.append(mk_k(c))
                    for i in range(4):
                        cls.append(mk_v(c, i))
                    cls.append(mk_q(c))
                return cls  # 24 closures (6 per chunk), PE-ordered K,V...,Q

            def emit_s_exp(b, fc, tt):
                sp = s_ps_pool.tile([128, 2, 512], F32, tag="s")
                for j in range(2):
                    nc.tensor.matmul(
                        sp[:, j, :],
                        lhsT=kT_sb[
                            64 * j : 64 * (j + 1), b, 128 * tt : 128 * (tt + 1)
                        ],
                        rhs=qTp_sb[
                            64 * j : 64 * (j + 1), b, 512 * fc : 512 * (fc + 1)
                        ],
                        start=True,
                        stop=True,
                        tile_position=(64 * j, 0),
                    )
                pt = pt_pool.tile([128, 2, 512], BF16, tag="pt")
                nc.scalar.activation(
                    out=pt[:, 0, :],
                    in_=sp[:, 0, :],
                    func=mybir.ActivationFunctionType.Exp,
                    scale=float(DH) ** -0.5,
                )
                with nc.allow_low_precision("schraudolph bf16 exp"):
                    nc.vector.tensor_scalar(
                        out=pt[:, 1, :].bitcast(I16),
                        in0=sp[:, 1, :],
                        scalar1=ASCHR,
                        scalar2=BSCHR,
                        op0=mybir.AluOpType.mult,
                        op1=mybir.AluOpType.add,
                    )
                return pt

            def flush(b, fc, a_ps):
                stf = stg_pool.tile([65, 2, 512], BF16, tag="stf")
                with nc.allow_low_precision("bf16 a2a shard"):
                    if fc % 2 == 0:
                        nc.vector.tensor_copy(out=stf[:, :, :], in_=a_ps[:, :, :])
                    else:
                        nc.scalar.copy(out=stf[:, :, :], in_=a_ps[:, :, :])
                for h in range(2):
                    dst = 2 * fc + h
                    sl = np.s_[256 * h : 256 * (h + 1)]
                    nc.scalar.dma_start(
                        out=a2a_in[b][dst, 0:32768].rearrange(
                            "(j p n) -> p j n", j=2, p=64, n=256
                        ),
                        in_=stf[0:64, :, sl],
                    )
                    nc.scalar.dma_start(
                        out=a2a_in[b][dst, 32768:33280].rearrange(
                            "(o j n) -> o j n", o=1, j=2, n=256
                        ),
                        in_=stf[64:65, :, sl],
                    )

            def outproj_closures(b):
                """Post-A2A(b): reciprocal + broadcast + normalize-multiply +
                output projection, as closures."""
                cls = []
                den16 = den_pool.tile([16, 256], BF16, tag="den16")
                den16f = den_pool.tile([16, 256], F32, tag="den16f")
                den16r = den_pool.tile([16, 256], F32, tag="den16r")
                den16rb = den_pool.tile([16, 256], BF16, tag="den16rb")
                den2 = den_pool.tile([2, 8, 256], BF16, tag="den2")
                atg = atg_pool.tile([128, 8, 256], BF16, tag="atg")

                def gather():
                    for a in range(8):
                        nc.sync.dma_start(
                            out=den16[2 * a : 2 * a + 2, :],
                            in_=a2a_out[b][a, 32768:33280].rearrange(
                                "(j n) -> j n", j=2, n=256
                            ),
                        )
                    nc.scalar.dma_start(
                        out=atg[:, :, :],
                        in_=a2a_out[b][:, 0:32768].rearrange(
                            "a (r n) -> r a n", r=128, n=256
                        ),
                    )

                def recip():
                    nc.vector.tensor_copy(out=den16f[:, :], in_=den16[:, :])
                    nc.vector.reciprocal_approx_fast(
                        out=den16r[:, :], in_=den16f[:, :]
                    )
                    with nc.allow_low_precision("bf16 denom recip"):
                        nc.vector.tensor_copy(out=den16rb[:, :], in_=den16r[:, :])
                    for a in range(8):
                        nc.sync.dma_start(
                            out=den2[:, a, :], in_=den16rb[2 * a : 2 * a + 2, :]
                        )

                cls.append(gather)
                cls.append(recip)
                sts = [None] * 8

                def mk_bc(a):
                    def f():
                        bc = pj_ps.tile([128, 512], F32, tag="pj", name="bc")
                        nc.tensor.matmul(
                            bc[:, 0:256],
                            lhsT=onesb[:, :],
                            rhs=den2[:, a, :],
                            start=True,
                            stop=True,
                        )
                        st = st_pool.tile([128, 256], BF16, tag="st")
                        with nc.allow_low_precision("bf16 attn normalize"):
                            nc.vector.tensor_mul(
                                out=st[:, :], in0=atg[:, a, :], in1=bc[:, 0:256]
                            )
                        sts[a] = st
                    return f

                for a in range(8):
                    cls.append(mk_bc(a))

                def mk_o(ft, jj):
                    def f():
                        o_ps = pj_ps.tile([128, 512], F32, tag="pj", name="o")
                        for a in range(8):
                            nc.tensor.matmul(
                                o_ps[:, :],
                                lhsT=sts[a][:, 128 * ft : 128 * (ft + 1)],
                                rhs=wo_sb[:, a, 512 * jj : 512 * (jj + 1)],
                                start=(a == 0),
                                stop=(a == 7),
                            )
                        ot = ot_pool.tile([128, 512], F32, tag="ot")
                        if jj == 0:
                            nc.vector.tensor_copy(out=ot[:, :], in_=o_ps[:, :])
                        else:
                            nc.scalar.copy(out=ot[:, :], in_=o_ps[:, :])
                        nc.scalar.dma_start(
                            out=out[
                                256 * b + 128 * ft : 256 * b + 128 * (ft + 1),
                                512 * jj : 512 * (jj + 1),
                            ],
                            in_=ot[:, :],
                        )
                    return f

                for ft in range(2):
                    for jj in range(2):
                        cls.append(mk_o(ft, jj))
                return cls  # 14 closures

            def attention_batch(b, sched):
                """sched: dict step -> list of closures to emit at that step."""
                steps = [(fc, tt) for fc in range(FC) for tt in range(TT)]
                pts = {}
                pts[steps[0]] = emit_s_exp(b, *steps[0])
                a_tiles = {}
                for i, (fc, tt) in enumerate(steps):
                    if tt == 0:
                        a_tiles[fc] = a_ps_pool.tile(
                            [65, 2, 512], F32, tag="a", name="a_acc"
                        )
                    last = tt == TT - 1
                    if not last and i + 1 < len(steps):
                        pts[steps[i + 1]] = emit_s_exp(b, *steps[i + 1])
                    # side work lands between the S matmuls and the P@V so the
                    # PE stays busy while this step's exp finishes
                    for cl in sched.get(i, ()):
                        cl()
                    a_ps = a_tiles[fc]
                    pt = pts.pop((fc, tt))
                    for j in range(2):
                        nc.tensor.matmul(
                            a_ps[:, j, :],
                            lhsT=v_sb[:, b, tt, j, :],
                            rhs=pt[:, j, :],
                            start=(tt == 0),
                            stop=(tt == TT - 1),
                        )
                    if last:
                        # flush first so the a_ps staging copy leads the exp
                        # engines' queues; next fc's first PV then has its
                        # accumulator free with no stall.
                        flush(b, fc, a_ps)
                        del a_tiles[fc]
                        if i + 1 < len(steps):
                            pts[steps[i + 1]] = emit_s_exp(b, *steps[i + 1])
                for i in sorted(k for k in sched if k >= len(steps)):
                    for cl in sched[i]:
                        cl()

            def spread(closures, start, end):
                """Schedule closures evenly over steps [start, end)."""
                sched = {}
                n = len(closures)
                for idx, cl in enumerate(closures):
                    step = start + idx * (end - start) // n
                    sched.setdefault(step, []).append(cl)
                return sched

            # ---- phase structure ------------------------------------------
            q0, s0 = input_dmas(0)
            p0 = proj_closures(0, q0, s0, use_scalar_copies=False)
            # chunk 0 of batch 0 runs serially (ramp)
            for cl in p0[0:6]:
                cl()
            q1, s1 = input_dmas(1)
            p1 = proj_closures(1, q1, s1, use_scalar_copies=True)

            # b0 attention: b0 chunks 1-3 early (chunk c needed by step 4c),
            # then b1's projections spread over the rest.
            sched0 = {}
            for c in range(1, 4):
                base = p0[6 * c : 6 * c + 6]
                for idx, cl in enumerate(base):
                    sched0.setdefault(4 * (c - 1) + idx // 3, []).append(cl)
            for step, cls in spread(p1, 14, 62).items():
                sched0.setdefault(step, []).extend(cls)
            attention_batch(0, sched0)

            nc.gpsimd.collective_compute(
                "AllToAll",
                mybir.AluOpType.bypass,
                replica_groups=[[0, 1, 2, 3, 4, 5, 6, 7]],
                ins=[a2a_in[0].ap().opt()],
                outs=[a2a_out[0].ap().opt()],
            )

            # wo only needed from b0's output projection onward.
            nc.scalar.dma_start(
                out=wo_sb[:, :, :],
                in_=wo[:, :].rearrange("(a p) n -> p a n", p=128),
            )

            # b1 attention with b0's output projection interleaved late.
            op0 = outproj_closures(0)
            sched1 = spread(op0, 30, 62)
            attention_batch(1, sched1)

            nc.gpsimd.collective_compute(
                "AllToAll",
                mybir.AluOpType.bypass,
                replica_groups=[[0, 1, 2, 3, 4, 5, 6, 7]],
                ins=[a2a_in[1].ap().opt()],
                outs=[a2a_out[1].ap().opt()],
            )

            for cl in outproj_closures(1):
                cl()

    nc.compile()
    return nc


def _get_nc():
    if "nc" not in _CACHE:
        _CACHE["nc"] = _build()
    return _CACHE["nc"]


def _reference_fallback(query_input, source_input, bias, wq, wk, wv, wo):
    """Numpy fallback, only used if bias is unexpectedly nonzero."""
    q = np.einsum("bfh,hnd->bfnd", query_input, wq) * (DH**-0.5)
    k = np.einsum("bth,hnd->btnd", source_input, wk)
    v = np.einsum("bth,hnd->btnd", source_input, wv)
    logits = np.einsum("btnd,bfnd->bnft", k, q) + bias
    logits -= logits.max(axis=-1, keepdims=True)
    w = np.exp(logits)
    w /= w.sum(axis=-1, keepdims=True)
    attn = np.einsum("bnft,btnd->bfnd", w, v)
    return np.einsum("bfnd,ndh->bfh", attn, wo).astype(np.float32)


def _chunk_major(xT):
    """[B, HID, F] -> [B*4, 128, HT*512] with 8KB-contiguous lines:
    out[4b+c, p, a*512+n] = xT[b, a*128+p, 512c+n]."""
    o = np.empty((B * 4, 128, HT * 512), dtype=NPBF16)
    for b in range(B):
        for c in range(4):
            chunk = xT[b, :, 512 * c : 512 * (c + 1)]  # [1024, 512]
            o[4 * b + c] = (
                chunk.reshape(HT, 128, 512).transpose(1, 0, 2).reshape(128, HT * 512)
            )
    return np.ascontiguousarray(o)


def make_in_maps(query_input, source_input, wq, wk, wv, wo):
    wo2 = np.ascontiguousarray(wo.reshape(HID, HID).astype(NPBF16))
    qTb = _chunk_major(np.transpose(query_input, (0, 2, 1)).astype(NPBF16))
    sTb = _chunk_major(np.transpose(source_input, (0, 2, 1)).astype(NPBF16))
    wqh = wq.reshape(HID, NH, DH)
    wkh = wk.reshape(HID, NH, DH)
    wvh = wv.reshape(HID, NH, DH)

    in_maps = []
    for c in range(8):
        sl = np.s_[:, 2 * c : 2 * c + 2, :]
        w3c = np.concatenate(
            [
                wqh[sl].reshape(HID, 128),
                wkh[sl].reshape(HID, 128),
                wvh[sl].reshape(HID, 128),
            ],
            axis=1,
        )  # [1024, 384]
        w3c = (
            w3c.reshape(HT, 128, 3, 128).transpose(1, 0, 2, 3).reshape(128, 3072)
        )
        in_maps.append(
            {
                "qT": qTb,
                "sT": sTb,
                "w3": np.ascontiguousarray(w3c).astype(NPBF16),
                "wo": wo2,
                "ob": OB_CONST,
            }
        )
    return in_maps


def assemble(results):
    """results[c]["out"] is [512, 1024]: rows 0-255 = batch 0 f-slice
    [256c, 256c+256), rows 256-511 = batch 1 same slice."""
    out_full = np.empty((B, F, HID), dtype=np.float32)
    for c in range(8):
        r = results[c]["out"]
        out_full[0, FS * c : FS * (c + 1), :] = r[0:FS]
        out_full[1, FS * c : FS * (c + 1), :] = r[FS : 2 * FS]
    return out_full


def kernel(query_input, source_input, bias, wq, wk, wv, wo):
    query_input = np.asarray(query_input, dtype=np.float32)
    source_input = np.asarray(source_input, dtype=np.float32)
    bias = np.asarray(bias, dtype=np.float32)
    wq = np.asarray(wq, dtype=np.float32)
    wk = np.asarray(wk, dtype=np.float32)
    wv = np.asarray(wv, dtype=np.float32)
    wo = np.asarray(wo, dtype=np.float32)

    if np.any(bias):
        return _reference_fallback(query_input, source_input, bias, wq, wk, wv, wo)

    in_maps = make_in_maps(query_input, source_input, wq, wk, wv, wo)
    nc = _get_nc()
    res = run_bass_kernel_spmd(nc, in_maps, core_ids=list(range(8)))
    return assemble(res.results)


# revision 16
# speedup vs baseline: 1.4926x; 1.0028x over previous
"""Multi-head attention (B=2, F=T=2048, H=1024, 16 heads x 64) on 8 TRN2
NeuronCores.

v3 design:
  * exp split across engines: per attention step, head j=0's exp runs on
    ACT (exact, free affine scale), head j=1's exp runs on DVE as a
    one-instruction Schraudolph bf16 exp (tensor_scalar mult+add with
    int16 output whose bits ARE the bf16 exp).
  * softmax normalization deferred to AFTER the AllToAll: shards carry 2
    denominator rows; denominators are inverted post-A2A with
    reciprocal_approx_fast on 16 partitions and broadcast via a tiny
    bf16 ones-matmul, folded into the bf16 conversion multiply.
  * AllToAll split per batch; A2A(b0) overlaps b1's attention. Output
    resharding: core c produces f-rows [256c, 256c+256) of BOTH batches.
  * work interleaving: batch-1 projections are emitted as side-closures
    inside batch-0's attention step loop (keeps PE dense/warm and the
    exp engines fed); batch-0's output projection is interleaved into
    batch-1's attention. Only b1's A2A + output projection are exposed.
  * inputs arrive chunk-major ([B*4, 128, HT*512]) so every DMA line is
    8KB contiguous.
"""

from contextlib import ExitStack

import ml_dtypes
import numpy as np

import concourse.bass as bass  # noqa: F401
import concourse.mybir as mybir
import concourse.tile as tile
from concourse import bacc
from concourse.bass_utils import run_bass_kernel_spmd

B, F, T, HID, NH, DH = 2, 2048, 2048, 1024, 16, 64
HT = HID // 128  # 8 h-tiles
TT = T // 128  # 16 key tiles
FC = F // 512  # 4 query chunks per batch
FS = 256  # f-rows per core per batch
SEG = 130 * 256  # A2A shard: 128 A^T rows + 2 denom rows, 256 f-cols
BF16, F32 = mybir.dt.bfloat16, mybir.dt.float32
I16 = mybir.dt.int16
NPBF16 = ml_dtypes.bfloat16

# Schraudolph bf16 exp: bits(int16) = s * ASCHR + BSCHR, s = raw logits.
ASCHR = 16.0 / float(np.log(2.0))  # (128/ln2) * (1/8 logit scale)
BSCHR = 16250.375  # calibrated; rel err +-3.3%, robust to trunc/nearest

OB_CONST = np.tile(np.kron(np.eye(2), np.ones((1, 64))), (8, 1)).astype(NPBF16)

_CACHE: dict = {}


def _build():
    nc = bacc.Bacc("TRN2", target_bir_lowering=False, debug=False, num_devices=8)

    qT = nc.declare_dram_parameter("qT", [B * 4, 128, HT * 512], BF16, isOutput=False)
    sT = nc.declare_dram_parameter("sT", [B * 4, 128, HT * 512], BF16, isOutput=False)
    w3 = nc.declare_dram_parameter("w3", [128, 3072], BF16, isOutput=False)
    wo = nc.declare_dram_parameter("wo", [HID, HID], BF16, isOutput=False)
    ob = nc.declare_dram_parameter("ob", [16, 128], BF16, isOutput=False)
    out = nc.declare_dram_parameter("out", [2 * FS, HID], F32, isOutput=True)

    a2a_in = [nc.dram_tensor(f"a2a_in{b}", [8, SEG], BF16) for b in range(B)]
    a2a_out = [nc.dram_tensor(f"a2a_out{b}", [8, SEG], BF16) for b in range(B)]

    with tile.TileContext(nc) as tc, ExitStack() as ctx:
        persist = ctx.enter_context(tc.tile_pool(name="persist", bufs=1))
        kT_sb = persist.tile([128, B, T], BF16, tag="kT")
        v_sb = persist.tile([128, B, TT, 2, DH + 1], BF16, tag="v")
        qTp_sb = persist.tile([128, B, F], BF16, tag="qTp")
        wo_sb = persist.tile([128, HT, HID], BF16, tag="wo")
        w3_sb = persist.tile([128, HT, 3, 128], BF16, tag="w3")  # wq|wk|wv
        onesb = persist.tile([16, 128], BF16, tag="onesb")

        nc.vector.memset(v_sb[:, :, :, :, DH : DH + 1], 1.0)
        nc.sync.dma_start(out=onesb[:, :], in_=ob[:, :])
        nc.sync.dma_start(
            out=w3_sb[:, :, :, :],
            in_=w3[:, :].rearrange("p (a k n) -> p a k n", a=HT, k=3),
        )

        with (
            tc.tile_pool(name="qin", bufs=4) as qin_pool,
            tc.tile_pool(name="sin", bufs=4) as sin_pool,
            tc.tile_pool(name="ptp", bufs=6) as pt_pool,
            tc.tile_pool(name="stg", bufs=3) as stg_pool,
            tc.tile_pool(name="den", bufs=2) as den_pool,
            tc.tile_pool(name="atg", bufs=2) as atg_pool,
            tc.tile_pool(name="stp", bufs=12) as st_pool,
            tc.tile_pool(name="otp", bufs=4) as ot_pool,
            tc.tile_pool(name="s_ps", bufs=2, space="PSUM") as s_ps_pool,
            tc.tile_pool(name="a_ps", bufs=1, space="PSUM") as a_ps_pool,
            tc.tile_pool(name="pj_ps", bufs=2, space="PSUM") as pj_ps,
        ):

            q_t = {0: [None] * 4, 1: [None] * 4}
            s_t = {0: [None] * 4, 1: [None] * 4}

            def dma_in(kind, b, c):
                if kind == "q":
                    qc = qin_pool.tile([128, HT, 512], BF16, tag="qc")
                    nc.sync.dma_start(
                        out=qc[:, :, :],
                        in_=qT[4 * b + c, :, :].rearrange("p (a n) -> p a n", a=HT),
                    )
                    q_t[b][c] = qc
                else:
                    sc = sin_pool.tile([128, HT, 512], BF16, tag="sc")
                    nc.sync.dma_start(
                        out=sc[:, :, :],
                        in_=sT[4 * b + c, :, :].rearrange("p (a n) -> p a n", a=HT),
                    )
                    s_t[b][c] = sc

            def proj_closures(b, qs, ss, use_scalar_copies):
                """Per-chunk projection closures (each emits PE matmuls and
                one PSUM->SBUF copy)."""
                cls = []

                def qk_copy(dst, src):
                    if use_scalar_copies:
                        nc.scalar.copy(out=dst, in_=src)
                    else:
                        nc.vector.tensor_copy(out=dst, in_=src)

                def mk_k(c):
                    def f():
                        ps = pj_ps.tile([128, 512], F32, tag="pj", name="ps_k")
                        for ht in range(HT):
                            nc.tensor.matmul(
                                ps[:, :],
                                lhsT=w3_sb[:, ht, 1, :],
                                rhs=ss[c][:, ht, :],
                                start=(ht == 0),
                                stop=(ht == HT - 1),
                            )
                        qk_copy(kT_sb[:, b, 512 * c : 512 * (c + 1)], ps[:, :])
                    return f

                def mk_v(c, i):
                    def f():
                        tt = 4 * c + i
                        ps = pj_ps.tile([128, 512], F32, tag="pj", name="ps_v")
                        for ht in range(HT):
                            nc.tensor.matmul(
                                ps[:, 0:128],
                                lhsT=ss[c][:, ht, 128 * i : 128 * (i + 1)],
                                rhs=w3_sb[:, ht, 2, :],
                                start=(ht == 0),
                                stop=(ht == HT - 1),
                            )
                        nc.vector.tensor_copy(
                            out=v_sb[:, b, tt, :, 0:DH],
                            in_=ps[:, 0:128].rearrange("p (j d) -> p j d", j=2),
                        )
                    return f

                def mk_q(c):
                    def f():
                        ps = pj_ps.tile([128, 512], F32, tag="pj", name="ps_q")
                        for ht in range(HT):
                            nc.tensor.matmul(
                                ps[:, :],
                                lhsT=w3_sb[:, ht, 0, :],
                                rhs=qs[c][:, ht, :],
                                start=(ht == 0),
                                stop=(ht == HT - 1),
                            )
                        qk_copy(qTp_sb[:, b, 512 * c : 512 * (c + 1)], ps[:, :])
                    return f

                for c in range(4):
                    cls.append(mk_k(c))
                    for i in range(4):
                        cls.append(mk_v(c, i))
                    cls.append(mk_q(c))
                return cls  # 24 closures (6 per chunk), PE-ordered K,V...,Q

            def emit_s_exp(b, fc, tt):
                sp = s_ps_pool.tile([128, 2, 512], F32, tag="s")
                for j in range(2):
                    nc.tensor.matmul(
                        sp[:, j, :],
                        lhsT=kT_sb[
                            64 * j : 64 * (j + 1), b, 128 * tt : 128 * (tt + 1)
                        ],
                        rhs=qTp_sb[
                            64 * j : 64 * (j + 1), b, 512 * fc : 512 * (fc + 1)
                        ],
                        start=True,
                        stop=True,
                        tile_position=(64 * j, 0),
                    )
                pt = pt_pool.tile([128, 2, 512], BF16, tag="pt")
                nc.scalar.activation(
                    out=pt[:, 0, :],
                    in_=sp[:, 0, :],
                    func=mybir.ActivationFunctionType.Exp,
                    scale=float(DH) ** -0.5,
                )
                with nc.allow_low_precision("schraudolph bf16 exp"):
                    nc.vector.tensor_scalar(
                        out=pt[:, 1, :].bitcast(I16),
                        in0=sp[:, 1, :],
                        scalar1=ASCHR,
                        scalar2=BSCHR,
                        op0=mybir.AluOpType.mult,
                        op1=mybir.AluOpType.add,
                    )
                return pt

            def flush(b, fc, a_ps):
                stf = stg_pool.tile([65, 2, 512], BF16, tag="stf")
                with nc.allow_low_precision("bf16 a2a shard"):
                    if fc % 2 == 0:
                        nc.vector.tensor_copy(out=stf[:, :, :], in_=a_ps[:, :, :])
                    else:
                        nc.scalar.copy(out=stf[:, :, :], in_=a_ps[:, :, :])
                for h in range(2):
                    dst = 2 * fc + h
                    sl = np.s_[256 * h : 256 * (h + 1)]
                    nc.scalar.dma_start(
                        out=a2a_in[b][dst, 0:32768].rearrange(
                            "(j p n) -> p j n", j=2, p=64, n=256
                        ),
                        in_=stf[0:64, :, sl],
                    )
                    nc.scalar.dma_start(
                        out=a2a_in[b][dst, 32768:33280].rearrange(
                            "(o j n) -> o j n", o=1, j=2, n=256
                        ),
                        in_=stf[64:65, :, sl],
                    )

            def outproj_closures(b):
                """Post-A2A(b): reciprocal + broadcast + normalize-multiply +
                output projection, as closures."""
                cls = []
                den16 = den_pool.tile([16, 256], BF16, tag="den16")
                den16f = den_pool.tile([16, 256], F32, tag="den16f")
                den16r = den_pool.tile([16, 256], F32, tag="den16r")
                den16rb = den_pool.tile([16, 256], BF16, tag="den16rb")
                den2 = den_pool.tile([2, 8, 256], BF16, tag="den2")
                atg = atg_pool.tile([128, 8, 256], BF16, tag="atg")

                def gather():
                    for a in range(8):
                        nc.sync.dma_start(
                            out=den16[2 * a : 2 * a + 2, :],
                            in_=a2a_out[b][a, 32768:33280].rearrange(
                                "(j n) -> j n", j=2, n=256
                            ),
                        )
                    nc.scalar.dma_start(
                        out=atg[:, :, :],
                        in_=a2a_out[b][:, 0:32768].rearrange(
                            "a (r n) -> r a n", r=128, n=256
                        ),
                    )

                def recip():
                    nc.vector.tensor_copy(out=den16f[:, :], in_=den16[:, :])
                    nc.vector.reciprocal_approx_fast(
                        out=den16r[:, :], in_=den16f[:, :]
                    )
                    with nc.allow_low_precision("bf16 denom recip"):
                        nc.vector.tensor_copy(out=den16rb[:, :], in_=den16r[:, :])
                    for a in range(8):
                        nc.sync.dma_start(
                            out=den2[:, a, :], in_=den16rb[2 * a : 2 * a + 2, :]
                        )

                cls.append(gather)
                cls.append(recip)
                sts = [None] * 8

                def mk_bc(a):
                    def f():
                        bc = pj_ps.tile([128, 512], F32, tag="pj", name="bc")
                        nc.tensor.matmul(
                            bc[:, 0:256],
                            lhsT=onesb[0:2, :],
                            rhs=den2[:, a, :],
                            start=True,
                            stop=True,
                        )
                        st = st_pool.tile([128, 256], BF16, tag="st")
                        with nc.allow_low_precision("bf16 attn normalize"):
                            nc.vector.tensor_mul(
                                out=st[:, :], in0=atg[:, a, :], in1=bc[:, 0:256]
                            )
                        sts[a] = st
                    return f

                for a in range(8):
                    cls.append(mk_bc(a))

                def mk_o(ft, jj):
                    def f():
                        o_ps = pj_ps.tile([128, 512], F32, tag="pj", name="o")
                        for a in range(8):
                            nc.tensor.matmul(
                                o_ps[:, :],
                                lhsT=sts[a][:, 128 * ft : 128 * (ft + 1)],
                                rhs=wo_sb[:, a, 512 * jj : 512 * (jj + 1)],
                                start=(a == 0),
                                stop=(a == 7),
                            )
                        ot = ot_pool.tile([128, 512], F32, tag="ot")
                        if jj == 0:
                            nc.vector.tensor_copy(out=ot[:, :], in_=o_ps[:, :])
                        else:
                            nc.scalar.copy(out=ot[:, :], in_=o_ps[:, :])
                        nc.scalar.dma_start(
                            out=out[
                                256 * b + 128 * ft : 256 * b + 128 * (ft + 1),
                                512 * jj : 512 * (jj + 1),
                            ],
                            in_=ot[:, :],
                        )
                    return f

                for ft in range(2):
                    for jj in range(2):
                        cls.append(mk_o(ft, jj))
                return cls  # 14 closures

            def attention_batch(b, sched):
                """sched: dict step -> list of closures to emit at that step."""
                steps = [(fc, tt) for fc in range(FC) for tt in range(TT)]
                pts = {}
                pts[steps[0]] = emit_s_exp(b, *steps[0])
                a_tiles = {}
                for i, (fc, tt) in enumerate(steps):
                    if tt == 0:
                        a_tiles[fc] = a_ps_pool.tile(
                            [65, 2, 512], F32, tag="a", name="a_acc"
                        )
                    last = tt == TT - 1
                    if not last and i + 1 < len(steps):
                        pts[steps[i + 1]] = emit_s_exp(b, *steps[i + 1])
                    # side work lands between the S matmuls and the P@V so the
                    # PE stays busy while this step's exp finishes
                    for cl in sched.get(i, ()):
                        cl()
                    a_ps = a_tiles[fc]
                    pt = pts.pop((fc, tt))
                    for j in range(2):
                        nc.tensor.matmul(
                            a_ps[:, j, :],
                            lhsT=v_sb[:, b, tt, j, :],
                            rhs=pt[:, j, :],
                            start=(tt == 0),
                            stop=(tt == TT - 1),
                        )
                    if last:
                        # flush first so the a_ps staging copy leads the exp
                        # engines' queues; next fc's first PV then has its
                        # accumulator free with no stall.
                        flush(b, fc, a_ps)
                        del a_tiles[fc]
                        if i + 1 < len(steps):
                            pts[steps[i + 1]] = emit_s_exp(b, *steps[i + 1])
                for i in sorted(k for k in sched if k >= len(steps)):
                    for cl in sched[i]:
                        cl()

            def spread(closures, start, end):
                """Schedule closures evenly over steps [start, end)."""
                sched = {}
                n = len(closures)
                for idx, cl in enumerate(closures):
                    step = start + idx * (end - start) // n
                    sched.setdefault(step, []).append(cl)
                return sched

            # ---- phase structure ------------------------------------------
            # Input DMAs in consumption-priority order (sync queue is FIFO).
            dma_in("s", 0, 0)
            dma_in("q", 0, 0)
            for c in range(1, 4):
                dma_in("s", 0, c)
            dma_in("q", 0, 1)
            dma_in("s", 1, 0)
            dma_in("q", 0, 2)
            dma_in("s", 1, 1)
            dma_in("q", 0, 3)
            dma_in("s", 1, 2)
            dma_in("s", 1, 3)
            for c in range(4):
                dma_in("q", 1, c)

            p0 = proj_closures(0, q_t[0], s_t[0], use_scalar_copies=False)
            # chunk 0 of batch 0 runs serially (ramp): K, V x4, Q
            for cl in p0[0:6]:
                cl()
            p1 = proj_closures(1, q_t[1], s_t[1], use_scalar_copies=True)

            # b0 attention schedule: K-c by step 4(c-1), V-c(tt) by step tt,
            # Q-c late (needed at step 16c); b1 K/V spread mid, Q last.
            sched0 = {}
            for c in range(1, 4):
                k_cl, v_cls, q_cl = p0[6 * c], p0[6 * c + 1 : 6 * c + 5], p0[6 * c + 5]
                sched0.setdefault(4 * (c - 1), []).append(k_cl)
                for i, cl in enumerate(v_cls):
                    sched0.setdefault(4 * (c - 1) + 1 + (3 * i) // 4, []).append(cl)
                sched0.setdefault(16 * c - 6, []).append(q_cl)
            b1_kv = []
            b1_q = []
            for c in range(4):
                b1_kv.append(p1[6 * c])
                b1_kv.extend(p1[6 * c + 1 : 6 * c + 5])
                b1_q.append(p1[6 * c + 5])
            for step, cls in spread(b1_kv, 16, 56).items():
                sched0.setdefault(step, []).extend(cls)
            for step, cls in spread(b1_q, 56, 64).items():
                sched0.setdefault(step, []).extend(cls)
            attention_batch(0, sched0)

            nc.gpsimd.collective_compute(
                "AllToAll",
                mybir.AluOpType.bypass,
                replica_groups=[[0, 1, 2, 3, 4, 5, 6, 7]],
                ins=[a2a_in[0].ap().opt()],
                outs=[a2a_out[0].ap().opt()],
            )

            # wo only needed from b0's output projection onward.
            nc.scalar.dma_start(
                out=wo_sb[:, :, :],
                in_=wo[:, :].rearrange("(a p) n -> p a n", p=128),
            )

            # b1 attention with b0's output projection interleaved late.
            op0 = outproj_closures(0)
            sched1 = spread(op0, 30, 62)
            attention_batch(1, sched1)

            nc.gpsimd.collective_compute(
                "AllToAll",
                mybir.AluOpType.bypass,
                replica_groups=[[0, 1, 2, 3, 4, 5, 6, 7]],
                ins=[a2a_in[1].ap().opt()],
                outs=[a2a_out[1].ap().opt()],
            )

            for cl in outproj_closures(1):
                cl()

    nc.compile()
    return nc


def _get_nc():
    if "nc" not in _CACHE:
        _CACHE["nc"] = _build()
    return _CACHE["nc"]


def _reference_fallback(query_input, source_input, bias, wq, wk, wv, wo):
    """Numpy fallback, only used if bias is unexpectedly nonzero."""
    q = np.einsum("bfh,hnd->bfnd", query_input, wq) * (DH**-0.5)
    k = np.einsum("bth,hnd->btnd", source_input, wk)
    v = np.einsum("bth,hnd->btnd", source_input, wv)
    logits = np.einsum("btnd,bfnd->bnft", k, q) + bias
    logits -= logits.max(axis=-1, keepdims=True)
    w = np.exp(logits)
    w /= w.sum(axis=-1, keepdims=True)
    attn = np.einsum("bnft,btnd->bfnd", w, v)
    return np.einsum("bfnd,ndh->bfh", attn, wo).astype(np.float32)


def _chunk_major(xT):
    """[B, HID, F] -> [B*4, 128, HT*512] with 8KB-contiguous lines:
    out[4b+c, p, a*512+n] = xT[b, a*128+p, 512c+n]."""
    o = np.empty((B * 4, 128, HT * 512), dtype=NPBF16)
    for b in range(B):
        for c in range(4):
            chunk = xT[b, :, 512 * c : 512 * (c + 1)]  # [1024, 512]
            o[4 * b + c] = (
                chunk.reshape(HT, 128, 512).transpose(1, 0, 2).reshape(128, HT * 512)
            )
    return np.ascontiguousarray(o)


def make_in_maps(query_input, source_input, wq, wk, wv, wo):
    wo2 = np.ascontiguousarray(wo.reshape(HID, HID).astype(NPBF16))
    qTb = _chunk_major(np.transpose(query_input, (0, 2, 1)).astype(NPBF16))
    sTb = _chunk_major(np.transpose(source_input, (0, 2, 1)).astype(NPBF16))
    wqh = wq.reshape(HID, NH, DH)
    wkh = wk.reshape(HID, NH, DH)
    wvh = wv.reshape(HID, NH, DH)

    in_maps = []
    for c in range(8):
        sl = np.s_[:, 2 * c : 2 * c + 2, :]
        w3c = np.concatenate(
            [
                wqh[sl].reshape(HID, 128),
                wkh[sl].reshape(HID, 128),
                wvh[sl].reshape(HID, 128),
            ],
            axis=1,
        )  # [1024, 384]
        w3c = (
            w3c.reshape(HT, 128, 3, 128).transpose(1, 0, 2, 3).reshape(128, 3072)
        )
        in_maps.append(
            {
                "qT": qTb,
                "sT": sTb,
                "w3": np.ascontiguousarray(w3c).astype(NPBF16),
                "wo": wo2,
                "ob": OB_CONST,
            }
        )
    return in_maps


def assemble(results):
    """results[c]["out"] is [512, 1024]: rows 0-255 = batch 0 f-slice
    [256c, 256c+256), rows 256-511 = batch 1 same slice."""
    out_full = np.empty((B, F, HID), dtype=np.float32)
    for c in range(8):
        r = results[c]["out"]
        out_full[0, FS * c : FS * (c + 1), :] = r[0:FS]
        out_full[1, FS * c : FS * (c + 1), :] = r[FS : 2 * FS]
    return out_full


def kernel(query_input, source_input, bias, wq, wk, wv, wo):
    query_input = np.asarray(query_input, dtype=np.float32)
    source_input = np.asarray(source_input, dtype=np.float32)
    bias = np.asarray(bias, dtype=np.float32)
    wq = np.asarray(wq, dtype=np.float32)
    wk = np.asarray(wk, dtype=np.float32)
    wv = np.asarray(wv, dtype=np.float32)
    wo = np.asarray(wo, dtype=np.float32)

    if np.any(bias):
        return _reference_fallback(query_input, source_input, bias, wq, wk, wv, wo)

    in_maps = make_in_maps(query_input, source_input, wq, wk, wv, wo)
    nc = _get_nc()
    res = run_bass_kernel_spmd(nc, in_maps, core_ids=list(range(8)))
    return assemble(res.results)
